# revision 17
# baseline (speedup 1.0000x reference)
"""Trainium2 Bass kernel for nn_CNN_PHMM_VAE loss (profile-HMM forward + VAE KLD).

Strategy: pure data parallel over 8 NeuronCores (64 examples per core).
The PHMM forward runs in probability space with periodic rescaling, so each
sequence step is pure multiply-adds on the vector engine:
  - delete-state column recurrence -> one tensor_tensor_scan (affine scan)
  - emission lookup -> cubic Horner in the symbol value (scalar_tensor_tensor)
  - state packed X = [mu | y] (y = mu + iota) so the insert-state update is a
    single 130-wide multiply against packed [G1-G2 | G2]
All per-example constant tables are precomputed on the host (O(B*K), ~0.4% of
the FLOPs); the O(B*L*K) dynamic program runs on device.

Layout per core: 64 examples on partitions, K+1=65 profile positions on the
free dimension.
"""
import numpy as np

B, L, K, E = 512, 256, 64, 16
K1 = K + 1
N_CORES = 8
BPC = B // N_CORES  # 64 examples per core
R = 64              # rescale interval (steps)
LOGACC0 = -60.0     # initial global log-scale
NEG = -100.0
M2M, M2I, M2D, I2M, I2I, D2M, D2D = 0, 1, 2, 3, 4, 5, 6

# --- big input table layout (free-dim offsets, per partition/example) -------
OFF_X0 = 0             # 130: [mu0 | y0]
OFF_U = 130            # 65 (U[0] = 0)
OFF_V = 195            # 65 (V[0] = 0)
OFF_GG = 260           # 130: [G1-G2 | G2]
OFF_H31 = 390          # 128: [H3 | H1]
OFF_H20 = 518          # 128: [H2 | H0]
OFF_S = 646            # 256 (symbols as float)
OFF_A1C0 = 902         # 1
OFF_A3C0 = 903         # 1
OFF_SIG0 = 904         # 1
OFF_LACC0 = 905        # 1
OFF_MUS = 906          # 16
OFF_LV = 922           # 16
TBL_W = 938

_CACHED = {}
XW = 132  # padded packed-state width: mu 0:65, pad, y 66:131, pad


def _host_tables(batch_input, transition_probs, emission_probs, mus, logvars):
    """Per-example constant tables, computed in float64, stored float32."""
    a = np.asarray(transition_probs, np.float64)
    Earr = np.exp(np.asarray(emission_probs, np.float64))  # (B, K, 4)
    A1 = np.exp(a[:, :, M2M])
    A2 = np.exp(a[:, :, I2M])
    A3 = np.exp(a[:, :, D2M])
    B1 = 0.25 * np.exp(a[:, :, M2I])
    B2 = 0.25 * np.exp(a[:, :, I2I])
    C1 = np.exp(a[:, :, M2D])
    C2 = np.exp(a[:, :, D2D])

    U = np.zeros((B, K1)); V = np.zeros((B, K1))
    U[:, 1:] = A3[:, 1:] * C1[:, :-1] / A1[:, :-1]
    V[:, 1:] = A3[:, 1:] * C2[:, :-1] / A3[:, :-1]
    G1 = A2 * B1 / A1
    G2 = B2

    # emission table gathered by symbol: etb[b, l*K+k] = A1[b,k+1] * E[b,k,s(b,l)]
    import ml_dtypes
    Etil = A1[:, 1:, None] * Earr  # (B, K, 4)
    s_idx = np.asarray(batch_input).astype(np.int64)  # (B, L)
    etb = np.take_along_axis(
        Etil.transpose(0, 2, 1),          # (B, 4, K)
        s_idx[:, :, None], axis=1)        # (B, L, K)
    etb = etb.reshape(B, L * K).astype(ml_dtypes.bfloat16)

    sig0 = np.exp(NEG - LOGACC0)          # e^-40, scaled NEG seed
    e0 = np.exp(-LOGACC0)                 # e^60, scaled "1.0"
    mu0 = np.empty((B, K1)); iot0 = np.empty((B, K1))
    mu0[:, 0] = A1[:, 0] * e0
    mu0[:, 1:] = A1[:, 1:] * sig0
    iot0[:, :] = A2 * sig0

    tbl = np.zeros((B, TBL_W), np.float32)
    tbl[:, OFF_X0:OFF_X0 + K1] = mu0
    tbl[:, OFF_X0 + K1:OFF_X0 + 2 * K1] = mu0 + iot0
    tbl[:, OFF_U:OFF_U + K1] = U
    tbl[:, OFF_V:OFF_V + K1] = V
    tbl[:, OFF_GG:OFF_GG + K1] = G1 - G2
    tbl[:, OFF_GG + K1:OFF_GG + 2 * K1] = G2
    tbl[:, OFF_A1C0] = A1[:, 0]
    tbl[:, OFF_A3C0] = A3[:, 0]
    tbl[:, OFF_SIG0] = sig0
    tbl[:, OFF_LACC0] = LOGACC0
    tbl[:, OFF_MUS:OFF_MUS + E] = np.asarray(mus, np.float32)
    tbl[:, OFF_LV:OFF_LV + E] = np.asarray(logvars, np.float32)
    return tbl, etb


def _make_in_maps(inputs):
    tbl, etb = _host_tables(**inputs)
    return [{"tbl": tbl[c * BPC:(c + 1) * BPC],
             "etb": etb[c * BPC:(c + 1) * BPC]} for c in range(N_CORES)]


def _build_bass(debug_taps=False):
    import concourse.bass as bass
    import concourse.tile as tile
    from concourse import bacc, mybir
    from contextlib import ExitStack

    f32 = mybir.dt.float32
    mult = mybir.AluOpType.mult
    add = mybir.AluOpType.add
    mx_op = mybir.AluOpType.max
    AF = mybir.ActivationFunctionType

    nc = bacc.Bacc("TRN2", target_bir_lowering=False, debug=False,
                   num_devices=N_CORES)
    bf = mybir.dt.bfloat16
    tbl_d = nc.dram_tensor("tbl", [BPC, TBL_W], f32, kind="ExternalInput").ap()
    etb_d = nc.dram_tensor("etb", [BPC, L * K], bf, kind="ExternalInput").ap()
    out_d = nc.dram_tensor("loss", [BPC, 1], f32, kind="ExternalOutput").ap()
    taps = {}
    if debug_taps:
        for nm in ["tf_o", "x_o"]:
            taps[nm] = nc.dram_tensor(nm, [BPC, 2 * K1], f32, kind="ExternalOutput").ap()
        for nm in ["delta_o"]:
            taps[nm] = nc.dram_tensor(nm, [BPC, K1], f32, kind="ExternalOutput").ap()
        for nm in ["lacc_o", "lnp_o", "nv_o", "kld_o", "sig_o"]:
            taps[nm] = nc.dram_tensor(nm, [BPC, 1], f32, kind="ExternalOutput").ap()

    with tile.TileContext(nc) as tc, ExitStack() as ctx:
        ctx.enter_context(nc.allow_low_precision(
            reason="bf16 DP state validated to ~2e-5 relative on the loss"))
        pool = ctx.enter_context(tc.tile_pool(name="p", bufs=1))

        TBL = pool.tile([BPC, TBL_W], f32, tag="TBL")
        nc.sync.dma_start(TBL[:, :], tbl_d[:, :])

        def tb(off, n):
            return TBL[:, off:off + n]

        U_ = tb(OFF_U, K1); V_ = tb(OFF_V, K1)
        GG_ = tb(OFF_GG, 2 * K1)

        # gathered emissions, streamed in 4 chunks of 64 steps each
        ECH = 64
        NCH = L // ECH
        e_ch = [pool.tile([BPC, ECH * K], bf, tag=f"ech{i}", name=f"ech{i}")
                for i in range(NCH)]
        for i in range(NCH):
            nc.sync.dma_start(e_ch[i][:, :],
                              etb_d[:, i * ECH * K:(i + 1) * ECH * K])

        # packed state X = [mu(65) | pad | y(65) | pad], bf16, ping-pong
        x_pp = [pool.tile([BPC, XW], bf, tag="x_a", name="x_a"),
                pool.tile([BPC, XW], bf, tag="x_b", name="x_b")]
        sig = pool.tile([BPC, 1], f32, tag="sig")
        beta = pool.tile([BPC, K1], bf, tag="beta")
        delta = pool.tile([BPC, K1], bf, tag="delta")
        t = pool.tile([BPC, K], bf, tag="t")
        ya_bufs = [pool.tile([BPC, K1 + 1], bf, tag=f"yab{i}", name=f"yab{i}")
                   for i in range(2)]
        r12_bufs = [pool.tile([BPC, XW], bf, tag=f"r12{i}", name=f"r12{i}")
                    for i in range(2)]
        Ub = pool.tile([BPC, K1], bf, tag="Ub")
        Vb = pool.tile([BPC, K1], bf, tag="Vb")
        GGb = pool.tile([BPC, XW], bf, tag="GGb")
        rmxb = pool.tile([BPC, 1], bf, tag="rmxb")

        v = nc.vector
        # KLD first: warms up the scalar engine so later activations carry
        # few sem waits (walrus limits waits per instruction)
        ev = pool.tile([BPC, E], f32, tag="ev")
        sq = pool.tile([BPC, E], f32, tag="sq")
        w1 = pool.tile([BPC, E], f32, tag="w1")
        w2 = pool.tile([BPC, E], f32, tag="w2")
        red = pool.tile([BPC, 1], f32, tag="red")
        kld = pool.tile([BPC, 1], f32, tag="kld")
        nc.scalar.activation(ev[:, :], tb(OFF_LV, E), AF.Exp)
        nc.scalar.activation(sq[:, :], tb(OFF_MUS, E), AF.Square)
        v.tensor_sub(w1[:, :], tb(OFF_LV, E), sq[:, :])
        v.tensor_sub(w2[:, :], w1[:, :], ev[:, :])
        v.tensor_reduce(red[:, :], w2[:, :], mybir.AxisListType.X, add)
        v.tensor_scalar(kld[:, :], red[:, :], -0.5, -float(E) / 2.0, mult, add)

        # init: cast constant tables to bf16, zero pads, init state
        v.memset(x_pp[0][:, :], 0.0)
        v.memset(x_pp[1][:, :], 0.0)
        v.memset(r12_bufs[0][:, :], 0.0)
        v.memset(r12_bufs[1][:, :], 0.0)
        v.tensor_copy(x_pp[0][:, 0:K1], tb(OFF_X0, K1))
        v.tensor_copy(x_pp[0][:, K1 + 1:2 * K1 + 1], tb(OFF_X0 + K1, K1))
        v.tensor_copy(Ub[:, :], U_)
        v.tensor_copy(Vb[:, :], V_)
        v.memset(GGb[:, :], 0.0)
        v.tensor_copy(GGb[:, 0:K1], tb(OFF_GG, K1))
        v.tensor_copy(GGb[:, K1 + 1:2 * K1 + 1], tb(OFF_GG + K1, K1))
        v.tensor_copy(sig[:, :], tb(OFF_SIG0, 1))
        v.tensor_mul(beta[:, 0:1], tb(OFF_A3C0, 1), sig[:, :])
        v.tensor_mul(x_pp[1][:, 0:1], tb(OFF_A1C0, 1), sig[:, :])

        YO = K1 + 1  # y offset in X (66, 4B-aligned for bf16)

        g = nc.gpsimd

        def dp_step(l):
            X, Xn = x_pp[l % 2], x_pp[(l + 1) % 2]
            ee = e_ch[l // ECH][:, (l % ECH) * K:(l % ECH + 1) * K]
            r12 = r12_bufs[l % 2]
            yab = ya_bufs[l % 2]
            # I-state product on gpsimd (reads only the previous state X)
            g.tensor_mul(r12[:, :], GGb[:, :], X[:, :])
            # M-chain (critical cycle) on DVE
            v.tensor_mul(beta[:, 1:K1], Ub[:, 1:K1], X[:, 0:K])
            v.tensor_tensor_scan(delta[:, :], Vb[:, :], beta[:, :], 0.0, mult, add)
            v.tensor_add(t[:, :], X[:, YO:YO + K], delta[:, 0:K])
            v.tensor_mul(Xn[:, 1:K1], ee, t[:, :])
            v.tensor_add(yab[:, :], r12[:, 0:K1 + 1], r12[:, YO:YO + K1 + 1])
            v.tensor_add(Xn[:, YO:YO + K1 + 1], Xn[:, 0:K1 + 1], yab[:, :])

        NRS = L // R - 1  # rescales (last one skipped)
        mxt = pool.tile([BPC, 1], f32, tag="mxt")
        rhist = pool.tile([BPC, NRS], f32, tag="rhist")

        def rescale(i, l):
            cur = (l + 1) % 2
            X = x_pp[cur]
            x_stale = x_pp[1 - cur]
            rmx = rhist[:, i:i + 1]
            # y >= mu and y >= iota elementwise, so max(y) is the state max
            v.tensor_reduce(mxt[:, :], X[:, YO:YO + K1], mybir.AxisListType.X, mx_op)
            v.reciprocal(rmxb[:, :], mxt[:, :])   # quantize factor to bf16
            v.tensor_copy(rmx, rmxb[:, :])        # record exact applied factor
            v.tensor_scalar_mul(X[:, :], X[:, :], rmx)
            v.tensor_scalar_mul(sig[:, :], sig[:, :], rmx)
            v.tensor_mul(beta[:, 0:1], tb(OFF_A3C0, 1), sig[:, :])
            v.tensor_mul(x_stale[:, 0:1], tb(OFF_A1C0, 1), sig[:, :])

        for l in range(L):
            dp_step(l)
            if l == 0:
                # column 0 of the l=0 buffer carried the "M0[0]=1" seed;
                # columns >= 1 reseed with sigma (the NEG=-100 re-injection)
                v.tensor_mul(x_pp[0][:, 0:1], tb(OFF_A1C0, 1), sig[:, :])
            if (l + 1) % R == 0 and (l + 1) < L:
                rescale((l + 1) // R - 1, l)

        # final column state: buffers L % 2 = 0
        Xf = x_pp[L % 2]
        tf = pool.tile([BPC, K1], f32, tag="tf")
        lnp = pool.tile([BPC, 1], f32, tag="lnp")
        lnr = pool.tile([BPC, NRS], f32, tag="lnr")
        sumlr = pool.tile([BPC, 1], f32, tag="sumlr")
        lacc = pool.tile([BPC, 1], f32, tag="lacc")
        nv = pool.tile([BPC, 1], f32, tag="nv")
        v.tensor_mul(beta[:, 1:K1], Ub[:, 1:K1], Xf[:, 0:K])
        v.tensor_tensor_scan(delta[:, :], Vb[:, :], beta[:, :], 0.0, mult, add)
        v.tensor_add(tf[:, :], Xf[:, YO:YO + K1], delta[:, :])
        nc.scalar.activation(lnp[:, :], tf[:, K:K1], AF.Ln)
        # lacc = LOGACC0 - sum_i ln(rmx_i)
        nc.scalar.activation(lnr[:, :], rhist[:, :], AF.Ln)
        v.tensor_reduce(sumlr[:, :], lnr[:, :], mybir.AxisListType.X, add)
        v.tensor_scalar(lacc[:, :], sumlr[:, :], -1.0, LOGACC0, mult, add)
        v.tensor_add(nv[:, :], lnp[:, :], lacc[:, :])  # = -nll
        loss_t = pool.tile([BPC, 1], f32, tag="loss_t")
        v.tensor_sub(loss_t[:, :], kld[:, :], nv[:, :])  # kld + nll
        nc.sync.dma_start(out_d[:, :], loss_t[:, :])
        if debug_taps:
            nc.sync.dma_start(taps["tf_o"][:, 0:K1], tf[:, :])
            nc.sync.dma_start(taps["x_o"], Xf[:, 0:2 * K1])
            nc.sync.dma_start(taps["delta_o"], delta[:, :])
            nc.sync.dma_start(taps["lacc_o"], lacc[:, :])
            nc.sync.dma_start(taps["lnp_o"], lnp[:, :])
            nc.sync.dma_start(taps["nv_o"], nv[:, :])
            nc.sync.dma_start(taps["kld_o"], kld[:, :])
            nc.sync.dma_start(taps["sig_o"], sig[:, :])

    nc.compile()
    return nc


def _get_nc():
    if "nc" not in _CACHED:
        _CACHED["nc"] = _build_bass()
    return _CACHED["nc"]


def kernel(batch_input, transition_probs, emission_probs, mus, logvars):
    from concourse.bass_utils import run_bass_kernel_spmd

    in_maps = _make_in_maps(dict(
        batch_input=batch_input, transition_probs=transition_probs,
        emission_probs=emission_probs, mus=mus, logvars=logvars))
    nc = _get_nc()
    res = run_bass_kernel_spmd(nc, in_maps, list(range(N_CORES)))
    losses = np.concatenate([np.asarray(r["loss"])[:, 0] for r in res.results])
    return np.float32(np.mean(losses.astype(np.float64)))



# revision 18
# speedup vs baseline: 1.2424x; 1.2424x over previous
"""Trainium2 Bass kernel for nn_CNN_PHMM_VAE loss (profile-HMM forward + VAE KLD).

Strategy: pure data parallel over 8 NeuronCores (64 examples per core).
The PHMM forward runs in probability space with periodic rescaling, so each
sequence step is pure multiply-adds on the vector engine:
  - delete-state column recurrence -> one tensor_tensor_scan (affine scan)
  - emission lookup -> cubic Horner in the symbol value (scalar_tensor_tensor)
  - state packed X = [mu | y] (y = mu + iota) so the insert-state update is a
    single 130-wide multiply against packed [G1-G2 | G2]
All per-example constant tables are precomputed on the host (O(B*K), ~0.4% of
the FLOPs); the O(B*L*K) dynamic program runs on device.

Layout per core: 64 examples on partitions, K+1=65 profile positions on the
free dimension.
"""
import numpy as np

B, L, K, E = 512, 256, 64, 16
K1 = K + 1
N_CORES = 8
BPC = B // N_CORES  # 64 examples per core
R = 64              # rescale interval (steps)
LOGACC0 = -60.0     # initial global log-scale
NEG = -100.0
M2M, M2I, M2D, I2M, I2I, D2M, D2D = 0, 1, 2, 3, 4, 5, 6

# --- big input table layout (free-dim offsets, per partition/example) -------
OFF_X0 = 0             # 130: [mu0 | y0]
OFF_U = 130            # 65 (U[0] = 0)
OFF_V = 195            # 65 (V[0] = 0)
OFF_GG = 260           # 130: [G1-G2 | G2]
OFF_H31 = 390          # 128: [H3 | H1]
OFF_H20 = 518          # 128: [H2 | H0]
OFF_S = 646            # 256 (symbols as float)
OFF_A1C0 = 902         # 1
OFF_A3C0 = 903         # 1
OFF_SIG0 = 904         # 1
OFF_LACC0 = 905        # 1
OFF_MUS = 906          # 16
OFF_LV = 922           # 16
TBL_W = 938

_CACHED = {}
XW = 132  # padded packed-state width: mu 0:65, pad, y 66:131, pad


def _host_tables(batch_input, transition_probs, emission_probs, mus, logvars):
    """Per-example constant tables, computed in float64, stored float32."""
    a = np.asarray(transition_probs, np.float64)
    Earr = np.exp(np.asarray(emission_probs, np.float64))  # (B, K, 4)
    A1 = np.exp(a[:, :, M2M])
    A2 = np.exp(a[:, :, I2M])
    A3 = np.exp(a[:, :, D2M])
    B1 = 0.25 * np.exp(a[:, :, M2I])
    B2 = 0.25 * np.exp(a[:, :, I2I])
    C1 = np.exp(a[:, :, M2D])
    C2 = np.exp(a[:, :, D2D])

    U = np.zeros((B, K1)); V = np.zeros((B, K1))
    U[:, 1:] = A3[:, 1:] * C1[:, :-1] / A1[:, :-1]
    V[:, 1:] = A3[:, 1:] * C2[:, :-1] / A3[:, :-1]
    G1 = A2 * B1 / A1
    G2 = B2

    # emission table gathered by symbol: etb[b, l*K+k] = A1[b,k+1] * E[b,k,s(b,l)]
    import ml_dtypes
    Etil = A1[:, 1:, None] * Earr  # (B, K, 4)
    s_idx = np.asarray(batch_input).astype(np.int64)  # (B, L)
    etb = np.take_along_axis(
        Etil.transpose(0, 2, 1),          # (B, 4, K)
        s_idx[:, :, None], axis=1)        # (B, L, K)
    etb = etb.reshape(B, L * K).astype(ml_dtypes.bfloat16)

    sig0 = np.exp(NEG - LOGACC0)          # e^-40, scaled NEG seed
    e0 = np.exp(-LOGACC0)                 # e^60, scaled "1.0"
    mu0 = np.empty((B, K1)); iot0 = np.empty((B, K1))
    mu0[:, 0] = A1[:, 0] * e0
    mu0[:, 1:] = A1[:, 1:] * sig0
    iot0[:, :] = A2 * sig0

    tbl = np.zeros((B, TBL_W), np.float32)
    tbl[:, OFF_X0:OFF_X0 + K1] = mu0
    tbl[:, OFF_X0 + K1:OFF_X0 + 2 * K1] = mu0 + iot0
    tbl[:, OFF_U:OFF_U + K1] = U
    tbl[:, OFF_V:OFF_V + K1] = V
    tbl[:, OFF_GG:OFF_GG + K1] = G1 - G2
    tbl[:, OFF_GG + K1:OFF_GG + 2 * K1] = G2
    tbl[:, OFF_A1C0] = A1[:, 0]
    tbl[:, OFF_A3C0] = A3[:, 0]
    tbl[:, OFF_SIG0] = sig0
    tbl[:, OFF_LACC0] = LOGACC0
    tbl[:, OFF_MUS:OFF_MUS + E] = np.asarray(mus, np.float32)
    tbl[:, OFF_LV:OFF_LV + E] = np.asarray(logvars, np.float32)
    return tbl, etb


def _make_in_maps(inputs):
    tbl, etb = _host_tables(**inputs)
    return [{"tbl": tbl[c * BPC:(c + 1) * BPC],
             "etb": etb[c * BPC:(c + 1) * BPC]} for c in range(N_CORES)]


def _build_bass(debug_taps=False):
    import concourse.bass as bass
    import concourse.tile as tile
    from concourse import bacc, mybir
    from contextlib import ExitStack

    f32 = mybir.dt.float32
    mult = mybir.AluOpType.mult
    add = mybir.AluOpType.add
    mx_op = mybir.AluOpType.max
    AF = mybir.ActivationFunctionType

    nc = bacc.Bacc("TRN2", target_bir_lowering=False, debug=False,
                   num_devices=N_CORES)
    bf = mybir.dt.bfloat16
    tbl_d = nc.dram_tensor("tbl", [BPC, TBL_W], f32, kind="ExternalInput").ap()
    etb_d = nc.dram_tensor("etb", [BPC, L * K], bf, kind="ExternalInput").ap()
    out_d = nc.dram_tensor("loss", [BPC, 1], f32, kind="ExternalOutput").ap()
    taps = {}
    if debug_taps:
        for nm in ["tf_o", "x_o"]:
            taps[nm] = nc.dram_tensor(nm, [BPC, 2 * K1], f32, kind="ExternalOutput").ap()
        for nm in ["delta_o"]:
            taps[nm] = nc.dram_tensor(nm, [BPC, K1], f32, kind="ExternalOutput").ap()
        for nm in ["lacc_o", "lnp_o", "nv_o", "kld_o", "sig_o"]:
            taps[nm] = nc.dram_tensor(nm, [BPC, 1], f32, kind="ExternalOutput").ap()

    with tile.TileContext(nc) as tc, ExitStack() as ctx:
        ctx.enter_context(nc.allow_low_precision(
            reason="bf16 DP state validated to ~2e-5 relative on the loss"))
        pool = ctx.enter_context(tc.tile_pool(name="p", bufs=1))

        TBL = pool.tile([BPC, TBL_W], f32, tag="TBL")
        nc.sync.dma_start(TBL[:, :], tbl_d[:, :])

        def tb(off, n):
            return TBL[:, off:off + n]

        U_ = tb(OFF_U, K1); V_ = tb(OFF_V, K1)
        GG_ = tb(OFF_GG, 2 * K1)

        # gathered emissions, streamed in 4 chunks of 64 steps each
        ECH = 64
        NCH = L // ECH
        e_ch = [pool.tile([BPC, ECH * K], bf, tag=f"ech{i}", name=f"ech{i}")
                for i in range(NCH)]
        for i in range(NCH):
            nc.sync.dma_start(e_ch[i][:, :],
                              etb_d[:, i * ECH * K:(i + 1) * ECH * K])

        # packed state X = [mu(65) | pad | y(65) | pad], bf16, ping-pong
        x_pp = [pool.tile([BPC, XW], bf, tag="x_a", name="x_a"),
                pool.tile([BPC, XW], bf, tag="x_b", name="x_b")]
        sig = pool.tile([BPC, 1], f32, tag="sig")
        beta = pool.tile([BPC, K1], bf, tag="beta")
        delta = pool.tile([BPC, K1], bf, tag="delta")
        t = pool.tile([BPC, K], bf, tag="t")
        ya_bufs = [pool.tile([BPC, K1 + 1], bf, tag=f"yab{i}", name=f"yab{i}")
                   for i in range(2)]
        r12_bufs = [pool.tile([BPC, XW], bf, tag=f"r12{i}", name=f"r12{i}")
                    for i in range(2)]
        Ub = pool.tile([BPC, K1], bf, tag="Ub")
        Vb = pool.tile([BPC, K1], bf, tag="Vb")
        GGb = pool.tile([BPC, XW], bf, tag="GGb")
        rmxb = pool.tile([BPC, 1], bf, tag="rmxb")

        v = nc.vector
        # KLD first: warms up the scalar engine so later activations carry
        # few sem waits (walrus limits waits per instruction)
        ev = pool.tile([BPC, E], f32, tag="ev")
        sq = pool.tile([BPC, E], f32, tag="sq")
        w1 = pool.tile([BPC, E], f32, tag="w1")
        w2 = pool.tile([BPC, E], f32, tag="w2")
        red = pool.tile([BPC, 1], f32, tag="red")
        kld = pool.tile([BPC, 1], f32, tag="kld")
        nc.scalar.activation(ev[:, :], tb(OFF_LV, E), AF.Exp)
        nc.scalar.activation(sq[:, :], tb(OFF_MUS, E), AF.Square)
        v.tensor_sub(w1[:, :], tb(OFF_LV, E), sq[:, :])
        v.tensor_sub(w2[:, :], w1[:, :], ev[:, :])
        v.tensor_reduce(red[:, :], w2[:, :], mybir.AxisListType.X, add)
        v.tensor_scalar(kld[:, :], red[:, :], -0.5, -float(E) / 2.0, mult, add)

        # init: cast constant tables to bf16, zero pads, init state
        v.memset(x_pp[0][:, :], 0.0)
        v.memset(x_pp[1][:, :], 0.0)
        v.memset(r12_bufs[0][:, :], 0.0)
        v.memset(r12_bufs[1][:, :], 0.0)
        v.tensor_copy(x_pp[0][:, 0:K1], tb(OFF_X0, K1))
        v.tensor_copy(x_pp[0][:, K1 + 1:2 * K1 + 1], tb(OFF_X0 + K1, K1))
        v.tensor_copy(Ub[:, :], U_)
        v.tensor_copy(Vb[:, :], V_)
        v.memset(GGb[:, :], 0.0)
        v.tensor_copy(GGb[:, 0:K1], tb(OFF_GG, K1))
        v.tensor_copy(GGb[:, K1 + 1:2 * K1 + 1], tb(OFF_GG + K1, K1))
        v.tensor_copy(sig[:, :], tb(OFF_SIG0, 1))
        v.tensor_mul(beta[:, 0:1], tb(OFF_A3C0, 1), sig[:, :])
        v.tensor_mul(x_pp[1][:, 0:1], tb(OFF_A1C0, 1), sig[:, :])

        YO = K1 + 1  # y offset in X (66, 4B-aligned for bf16)

        def dp_step(l):
            X, Xn = x_pp[l % 2], x_pp[(l + 1) % 2]
            ee = e_ch[l // ECH][:, (l % ECH) * K:(l % ECH + 1) * K]
            r12 = r12_bufs[l % 2]
            yab = ya_bufs[l % 2]
            v.tensor_mul(beta[:, 1:K1], Ub[:, 1:K1], X[:, 0:K])
            v.tensor_tensor_scan(delta[:, :], Vb[:, :], beta[:, :], 0.0, mult, add)
            v.tensor_add(t[:, :], X[:, YO:YO + K], delta[:, 0:K])
            v.tensor_mul(r12[:, :], GGb[:, :], X[:, :])
            v.tensor_mul(Xn[:, 1:K1], ee, t[:, :])
            v.tensor_add(yab[:, :], r12[:, 0:K1 + 1], r12[:, YO:YO + K1 + 1])
            v.tensor_add(Xn[:, YO:YO + K1 + 1], Xn[:, 0:K1 + 1], yab[:, :])

        NRS = L // R - 1  # rescales (last one skipped)
        mxt = pool.tile([BPC, 1], f32, tag="mxt")
        rhist = pool.tile([BPC, NRS], f32, tag="rhist")

        def rescale(i, l):
            cur = (l + 1) % 2
            X = x_pp[cur]
            x_stale = x_pp[1 - cur]
            rmx = rhist[:, i:i + 1]
            # y >= mu and y >= iota elementwise, so max(y) is the state max
            v.tensor_reduce(mxt[:, :], X[:, YO:YO + K1], mybir.AxisListType.X, mx_op)
            v.reciprocal(rmxb[:, :], mxt[:, :])   # quantize factor to bf16
            v.tensor_copy(rmx, rmxb[:, :])        # record exact applied factor
            v.tensor_scalar_mul(X[:, :], X[:, :], rmx)
            v.tensor_scalar_mul(sig[:, :], sig[:, :], rmx)
            v.tensor_mul(beta[:, 0:1], tb(OFF_A3C0, 1), sig[:, :])
            v.tensor_mul(x_stale[:, 0:1], tb(OFF_A1C0, 1), sig[:, :])

        for l in range(L):
            dp_step(l)
            if l == 0:
                # column 0 of the l=0 buffer carried the "M0[0]=1" seed;
                # columns >= 1 reseed with sigma (the NEG=-100 re-injection)
                v.tensor_mul(x_pp[0][:, 0:1], tb(OFF_A1C0, 1), sig[:, :])
            if (l + 1) % R == 0 and (l + 1) < L:
                rescale((l + 1) // R - 1, l)

        # final column state: buffers L % 2 = 0
        Xf = x_pp[L % 2]
        tf = pool.tile([BPC, K1], f32, tag="tf")
        lnp = pool.tile([BPC, 1], f32, tag="lnp")
        lnr = pool.tile([BPC, NRS], f32, tag="lnr")
        sumlr = pool.tile([BPC, 1], f32, tag="sumlr")
        lacc = pool.tile([BPC, 1], f32, tag="lacc")
        nv = pool.tile([BPC, 1], f32, tag="nv")
        v.tensor_mul(beta[:, 1:K1], Ub[:, 1:K1], Xf[:, 0:K])
        v.tensor_tensor_scan(delta[:, :], Vb[:, :], beta[:, :], 0.0, mult, add)
        v.tensor_add(tf[:, :], Xf[:, YO:YO + K1], delta[:, :])
        nc.scalar.activation(lnp[:, :], tf[:, K:K1], AF.Ln)
        # lacc = LOGACC0 - sum_i ln(rmx_i)
        nc.scalar.activation(lnr[:, :], rhist[:, :], AF.Ln)
        v.tensor_reduce(sumlr[:, :], lnr[:, :], mybir.AxisListType.X, add)
        v.tensor_scalar(lacc[:, :], sumlr[:, :], -1.0, LOGACC0, mult, add)
        v.tensor_add(nv[:, :], lnp[:, :], lacc[:, :])  # = -nll
        loss_t = pool.tile([BPC, 1], f32, tag="loss_t")
        v.tensor_sub(loss_t[:, :], kld[:, :], nv[:, :])  # kld + nll
        nc.sync.dma_start(out_d[:, :], loss_t[:, :])
        if debug_taps:
            nc.sync.dma_start(taps["tf_o"][:, 0:K1], tf[:, :])
            nc.sync.dma_start(taps["x_o"], Xf[:, 0:2 * K1])
            nc.sync.dma_start(taps["delta_o"], delta[:, :])
            nc.sync.dma_start(taps["lacc_o"], lacc[:, :])
            nc.sync.dma_start(taps["lnp_o"], lnp[:, :])
            nc.sync.dma_start(taps["nv_o"], nv[:, :])
            nc.sync.dma_start(taps["kld_o"], kld[:, :])
            nc.sync.dma_start(taps["sig_o"], sig[:, :])

    nc.compile()
    return nc


def _get_nc():
    if "nc" not in _CACHED:
        _CACHED["nc"] = _build_bass()
    return _CACHED["nc"]


def kernel(batch_input, transition_probs, emission_probs, mus, logvars):
    from concourse.bass_utils import run_bass_kernel_spmd

    in_maps = _make_in_maps(dict(
        batch_input=batch_input, transition_probs=transition_probs,
        emission_probs=emission_probs, mus=mus, logvars=logvars))
    nc = _get_nc()
    res = run_bass_kernel_spmd(nc, in_maps, list(range(N_CORES)))
    losses = np.concatenate([np.asarray(r["loss"])[:, 0] for r in res.results])
    return np.float32(np.mean(losses.astype(np.float64)))



# revision 20
# speedup vs baseline: 1.3880x; 1.1171x over previous
"""Trainium2 Bass kernel for nn_CNN_PHMM_VAE loss (profile-HMM forward + VAE KLD).

Strategy: pure data parallel over 8 NeuronCores (64 examples per core).
The PHMM forward runs in probability space with periodic rescaling, so each
sequence step is pure multiply-adds on the vector engine:
  - delete-state column recurrence -> one tensor_tensor_scan (affine scan)
  - emission lookup -> cubic Horner in the symbol value (scalar_tensor_tensor)
  - state packed X = [mu | y] (y = mu + iota) so the insert-state update is a
    single 130-wide multiply against packed [G1-G2 | G2]
All per-example constant tables are precomputed on the host (O(B*K), ~0.4% of
the FLOPs); the O(B*L*K) dynamic program runs on device.

Layout per core: 64 examples on partitions, K+1=65 profile positions on the
free dimension.
"""
import numpy as np

B, L, K, E = 512, 256, 64, 16
K1 = K + 1
N_CORES = 8
BPC = B // N_CORES  # 64 examples per core
R = 64              # rescale interval (steps)
LOGACC0 = -60.0     # initial global log-scale
NEG = -100.0
M2M, M2I, M2D, I2M, I2I, D2M, D2D = 0, 1, 2, 3, 4, 5, 6

# --- big input table layout (free-dim offsets, per partition/example) -------
OFF_X0 = 0             # 130: [mu0 | y0]
OFF_U = 130            # 65 (U[0] = 0)
OFF_V = 195            # 65 (V[0] = 0)
OFF_GG = 260           # 130: [G1-G2 | G2]
OFF_H31 = 390          # 128: [H3 | H1]
OFF_H20 = 518          # 128: [H2 | H0]
OFF_S = 646            # 256 (symbols as float)
OFF_A1C0 = 902         # 1
OFF_A3C0 = 903         # 1
OFF_SIG0 = 904         # 1
OFF_LACC0 = 905        # 1
OFF_MUS = 906          # 16
OFF_LV = 922           # 16
TBL_W = 938

_CACHED = {}
XW = 132  # padded packed-state width: mu 0:65, pad, y 66:131, pad


def _host_tables(batch_input, transition_probs, emission_probs, mus, logvars):
    """Per-example constant tables, computed in float64, stored float32."""
    a = np.asarray(transition_probs, np.float64)
    Earr = np.exp(np.asarray(emission_probs, np.float64))  # (B, K, 4)
    A1 = np.exp(a[:, :, M2M])
    A2 = np.exp(a[:, :, I2M])
    A3 = np.exp(a[:, :, D2M])
    B1 = 0.25 * np.exp(a[:, :, M2I])
    B2 = 0.25 * np.exp(a[:, :, I2I])
    C1 = np.exp(a[:, :, M2D])
    C2 = np.exp(a[:, :, D2D])

    U = np.zeros((B, K1)); V = np.zeros((B, K1))
    U[:, 1:] = A3[:, 1:] * C1[:, :-1] / A1[:, :-1]
    V[:, 1:] = A3[:, 1:] * C2[:, :-1] / A3[:, :-1]
    G1 = A2 * B1 / A1
    G2 = B2

    # emission table gathered by symbol: etb[b, l*K+k] = A1[b,k+1] * E[b,k,s(b,l)]
    import ml_dtypes
    Etil = A1[:, 1:, None] * Earr  # (B, K, 4)
    s_idx = np.asarray(batch_input).astype(np.int64)  # (B, L)
    etb = np.take_along_axis(
        Etil.transpose(0, 2, 1),          # (B, 4, K)
        s_idx[:, :, None], axis=1)        # (B, L, K)
    etb = etb.reshape(B, L * K).astype(ml_dtypes.bfloat16)

    sig0 = np.exp(NEG - LOGACC0)          # e^-40, scaled NEG seed
    e0 = np.exp(-LOGACC0)                 # e^60, scaled "1.0"
    mu0 = np.empty((B, K1)); iot0 = np.empty((B, K1))
    mu0[:, 0] = A1[:, 0] * e0
    mu0[:, 1:] = A1[:, 1:] * sig0
    iot0[:, :] = A2 * sig0

    tbl = np.zeros((B, TBL_W), np.float32)
    tbl[:, OFF_X0:OFF_X0 + K1] = mu0
    tbl[:, OFF_X0 + K1:OFF_X0 + 2 * K1] = mu0 + iot0
    tbl[:, OFF_U:OFF_U + K1] = U
    tbl[:, OFF_V:OFF_V + K1] = V
    tbl[:, OFF_GG:OFF_GG + K1] = G1 - G2
    tbl[:, OFF_GG + K1:OFF_GG + 2 * K1] = G2
    tbl[:, OFF_A1C0] = A1[:, 0]
    tbl[:, OFF_A3C0] = A3[:, 0]
    tbl[:, OFF_SIG0] = sig0
    tbl[:, OFF_LACC0] = LOGACC0
    tbl[:, OFF_MUS:OFF_MUS + E] = np.asarray(mus, np.float32)
    tbl[:, OFF_LV:OFF_LV + E] = np.asarray(logvars, np.float32)
    return tbl, etb


def _make_in_maps(inputs):
    tbl, etb = _host_tables(**inputs)
    return [{"tbl": tbl[c * BPC:(c + 1) * BPC],
             "etb": etb[c * BPC:(c + 1) * BPC]} for c in range(N_CORES)]


def _build_bass(debug_taps=False):
    import concourse.bass as bass
    import concourse.tile as tile
    from concourse import bacc, mybir
    from contextlib import ExitStack

    f32 = mybir.dt.float32
    mult = mybir.AluOpType.mult
    add = mybir.AluOpType.add
    mx_op = mybir.AluOpType.max
    AF = mybir.ActivationFunctionType

    nc = bacc.Bacc("TRN2", target_bir_lowering=False, debug=False,
                   num_devices=N_CORES)
    bf = mybir.dt.bfloat16
    tbl_d = nc.dram_tensor("tbl", [BPC, TBL_W], f32, kind="ExternalInput").ap()
    etb_d = nc.dram_tensor("etb", [BPC, L * K], bf, kind="ExternalInput").ap()
    out_d = nc.dram_tensor("loss", [BPC, 1], f32, kind="ExternalOutput").ap()
    taps = {}
    if debug_taps:
        for nm in ["tf_o", "x_o"]:
            taps[nm] = nc.dram_tensor(nm, [BPC, 2 * K1], f32, kind="ExternalOutput").ap()
        for nm in ["delta_o"]:
            taps[nm] = nc.dram_tensor(nm, [BPC, K1], f32, kind="ExternalOutput").ap()
        for nm in ["lacc_o", "lnp_o", "nv_o", "kld_o", "sig_o"]:
            taps[nm] = nc.dram_tensor(nm, [BPC, 1], f32, kind="ExternalOutput").ap()

    with tile.TileContext(nc) as tc, ExitStack() as ctx:
        ctx.enter_context(nc.allow_low_precision(
            reason="bf16 DP state validated to ~2e-5 relative on the loss"))
        pool = ctx.enter_context(tc.tile_pool(name="p", bufs=1))

        TBL = pool.tile([BPC, TBL_W], f32, tag="TBL")
        nc.sync.dma_start(TBL[:, :], tbl_d[:, :])

        def tb(off, n):
            return TBL[:, off:off + n]

        U_ = tb(OFF_U, K1); V_ = tb(OFF_V, K1)
        GG_ = tb(OFF_GG, 2 * K1)

        # gathered emissions, streamed in 4 chunks of 64 steps each
        ECH = 64
        NCH = L // ECH
        e_ch = [pool.tile([BPC, ECH * K], bf, tag=f"ech{i}", name=f"ech{i}")
                for i in range(NCH)]
        for i in range(NCH):
            nc.sync.dma_start(e_ch[i][:, :],
                              etb_d[:, i * ECH * K:(i + 1) * ECH * K])

        # packed state X = [mu(65) | pad | y(65) | pad], bf16, ping-pong
        x_pp = [pool.tile([BPC, XW], bf, tag="x_a", name="x_a"),
                pool.tile([BPC, XW], bf, tag="x_b", name="x_b")]
        sig = pool.tile([BPC, 1], f32, tag="sig")
        beta = pool.tile([BPC, K1], bf, tag="beta")
        delta = pool.tile([BPC, K1], bf, tag="delta")
        t = pool.tile([BPC, K], bf, tag="t")
        ya_bufs = [pool.tile([BPC, K1 + 1], bf, tag=f"yab{i}", name=f"yab{i}")
                   for i in range(2)]
        r12_bufs = [pool.tile([BPC, XW], bf, tag=f"r12{i}", name=f"r12{i}")
                    for i in range(2)]
        Ub = pool.tile([BPC, K1], bf, tag="Ub")
        Vb = pool.tile([BPC, K1], bf, tag="Vb")
        GGb = pool.tile([BPC, XW], bf, tag="GGb")
        rmxb = pool.tile([BPC, 1], bf, tag="rmxb")

        v = nc.vector
        # KLD first: warms up the scalar engine so later activations carry
        # few sem waits (walrus limits waits per instruction)
        ev = pool.tile([BPC, E], f32, tag="ev")
        sq = pool.tile([BPC, E], f32, tag="sq")
        w1 = pool.tile([BPC, E], f32, tag="w1")
        w2 = pool.tile([BPC, E], f32, tag="w2")
        red = pool.tile([BPC, 1], f32, tag="red")
        kld = pool.tile([BPC, 1], f32, tag="kld")
        nc.scalar.activation(ev[:, :], tb(OFF_LV, E), AF.Exp)
        nc.scalar.activation(sq[:, :], tb(OFF_MUS, E), AF.Square)
        v.tensor_sub(w1[:, :], tb(OFF_LV, E), sq[:, :])
        v.tensor_sub(w2[:, :], w1[:, :], ev[:, :])
        v.tensor_reduce(red[:, :], w2[:, :], mybir.AxisListType.X, add)
        v.tensor_scalar(kld[:, :], red[:, :], -0.5, -float(E) / 2.0, mult, add)

        # init: cast constant tables to bf16, zero pads, init state
        v.memset(x_pp[0][:, :], 0.0)
        v.memset(x_pp[1][:, :], 0.0)
        v.memset(r12_bufs[0][:, :], 0.0)
        v.memset(r12_bufs[1][:, :], 0.0)
        v.tensor_copy(x_pp[0][:, 0:K1], tb(OFF_X0, K1))
        v.tensor_copy(x_pp[0][:, K1 + 1:2 * K1 + 1], tb(OFF_X0 + K1, K1))
        v.tensor_copy(Ub[:, :], U_)
        v.tensor_copy(Vb[:, :], V_)
        v.memset(GGb[:, :], 0.0)
        v.tensor_copy(GGb[:, 0:K1], tb(OFF_GG, K1))
        v.tensor_copy(GGb[:, K1 + 1:2 * K1 + 1], tb(OFF_GG + K1, K1))
        v.tensor_copy(sig[:, :], tb(OFF_SIG0, 1))
        v.tensor_mul(beta[:, 0:1], tb(OFF_A3C0, 1), sig[:, :])
        v.tensor_mul(x_pp[1][:, 0:1], tb(OFF_A1C0, 1), sig[:, :])

        YO = K1 + 1  # y offset in X (66, 4B-aligned for bf16)

        def emit_beta(l):
            # beta(l) = U * mu(l-input-state); reads only the mu half of X(l)
            X = x_pp[l % 2]
            v.tensor_mul(beta[:, 1:K1], Ub[:, 1:K1], X[:, 0:K])

        def dp_step(l):
            # beta(l) was emitted by the previous iteration (software pipeline)
            # so scan can issue immediately; independent ops are interleaved
            # between dependent pairs to hide producer-complete latencies.
            X, Xn = x_pp[l % 2], x_pp[(l + 1) % 2]
            ee = e_ch[l // ECH][:, (l % ECH) * K:(l % ECH + 1) * K]
            r12 = r12_bufs[l % 2]
            yab = ya_bufs[l % 2]
            v.tensor_tensor_scan(delta[:, :], Vb[:, :], beta[:, :], 0.0, mult, add)
            v.tensor_mul(r12[:, :], GGb[:, :], X[:, :])
            v.tensor_add(t[:, :], X[:, YO:YO + K], delta[:, 0:K])
            v.tensor_add(yab[:, :], r12[:, 0:K1 + 1], r12[:, YO:YO + K1 + 1])
            v.tensor_mul(Xn[:, 1:K1], ee, t[:, :])
            if l + 1 < L:
                emit_beta(l + 1)
            v.tensor_add(Xn[:, YO:YO + K1 + 1], Xn[:, 0:K1 + 1], yab[:, :])

        NRS = L // R - 1  # rescales (last one skipped)
        mxt = pool.tile([BPC, 1], f32, tag="mxt")
        rhist = pool.tile([BPC, NRS], f32, tag="rhist")

        def rescale(i, l):
            cur = (l + 1) % 2
            X = x_pp[cur]
            x_stale = x_pp[1 - cur]
            rmx = rhist[:, i:i + 1]
            # y >= mu and y >= iota elementwise, so max(y) is the state max
            v.tensor_reduce(mxt[:, :], X[:, YO:YO + K1], mybir.AxisListType.X, mx_op)
            v.reciprocal(rmxb[:, :], mxt[:, :])   # quantize factor to bf16
            v.tensor_copy(rmx, rmxb[:, :])        # record exact applied factor
            v.tensor_scalar_mul(X[:, :], X[:, :], rmx)
            v.tensor_scalar_mul(sig[:, :], sig[:, :], rmx)
            v.tensor_mul(beta[:, 0:1], tb(OFF_A3C0, 1), sig[:, :])
            v.tensor_mul(x_stale[:, 0:1], tb(OFF_A1C0, 1), sig[:, :])
            emit_beta(l + 1)  # state was rescaled: redo the pipelined beta

        emit_beta(0)
        for l in range(L):
            dp_step(l)
            if l == 0:
                # column 0 of the l=0 buffer carried the "M0[0]=1" seed;
                # columns >= 1 reseed with sigma (the NEG=-100 re-injection)
                v.tensor_mul(x_pp[0][:, 0:1], tb(OFF_A1C0, 1), sig[:, :])
            if (l + 1) % R == 0 and (l + 1) < L:
                rescale((l + 1) // R - 1, l)

        # final column state: buffers L % 2 = 0
        Xf = x_pp[L % 2]
        tf = pool.tile([BPC, K1], f32, tag="tf")
        lnp = pool.tile([BPC, 1], f32, tag="lnp")
        lnr = pool.tile([BPC, NRS], f32, tag="lnr")
        sumlr = pool.tile([BPC, 1], f32, tag="sumlr")
        lacc = pool.tile([BPC, 1], f32, tag="lacc")
        nv = pool.tile([BPC, 1], f32, tag="nv")
        v.tensor_mul(beta[:, 1:K1], Ub[:, 1:K1], Xf[:, 0:K])
        v.tensor_tensor_scan(delta[:, :], Vb[:, :], beta[:, :], 0.0, mult, add)
        v.tensor_add(tf[:, :], Xf[:, YO:YO + K1], delta[:, :])
        nc.scalar.activation(lnp[:, :], tf[:, K:K1], AF.Ln)
        # lacc = LOGACC0 - sum_i ln(rmx_i)
        nc.scalar.activation(lnr[:, :], rhist[:, :], AF.Ln)
        v.tensor_reduce(sumlr[:, :], lnr[:, :], mybir.AxisListType.X, add)
        v.tensor_scalar(lacc[:, :], sumlr[:, :], -1.0, LOGACC0, mult, add)
        v.tensor_add(nv[:, :], lnp[:, :], lacc[:, :])  # = -nll
        loss_t = pool.tile([BPC, 1], f32, tag="loss_t")
        v.tensor_sub(loss_t[:, :], kld[:, :], nv[:, :])  # kld + nll
        nc.sync.dma_start(out_d[:, :], loss_t[:, :])
        if debug_taps:
            nc.sync.dma_start(taps["tf_o"][:, 0:K1], tf[:, :])
            nc.sync.dma_start(taps["x_o"], Xf[:, 0:2 * K1])
            nc.sync.dma_start(taps["delta_o"], delta[:, :])
            nc.sync.dma_start(taps["lacc_o"], lacc[:, :])
            nc.sync.dma_start(taps["lnp_o"], lnp[:, :])
            nc.sync.dma_start(taps["nv_o"], nv[:, :])
            nc.sync.dma_start(taps["kld_o"], kld[:, :])
            nc.sync.dma_start(taps["sig_o"], sig[:, :])

    nc.compile()
    return nc


def _get_nc():
    if "nc" not in _CACHED:
        _CACHED["nc"] = _build_bass()
    return _CACHED["nc"]


def kernel(batch_input, transition_probs, emission_probs, mus, logvars):
    from concourse.bass_utils import run_bass_kernel_spmd

    in_maps = _make_in_maps(dict(
        batch_input=batch_input, transition_probs=transition_probs,
        emission_probs=emission_probs, mus=mus, logvars=logvars))
    nc = _get_nc()
    res = run_bass_kernel_spmd(nc, in_maps, list(range(N_CORES)))
    losses = np.concatenate([np.asarray(r["loss"])[:, 0] for r in res.results])
    return np.float32(np.mean(losses.astype(np.float64)))

